# revision 24
# baseline (speedup 1.0000x reference)
"""Trainium2 Bass kernel for nn_Encoder_79843442033106 (retrieval_knn).

Reference computation:
  queries xq[b,k,:] (1024 x 2016, fp16 values) are matched against a codebook
  c (16001 x 2016) under squared L2 distance, searching the concatenation
  [d0, d1, d1, d0] where d0 = ||x-c||^2 and d1 = ||x-(1-c)||^2; the argmin
  index is emitted LSB-first as 32 bits -> output [64, 512] int32.

Identities used (per query q, code m; x2 = ||x||^2 is per-query and cannot
affect any argmin, so it is dropped everywhere):
  d0 - x2 =  c2[m] - 2*xc[q,m]          ( = -g0 )
  d1 - x2 = cn2[m] - 2*(xs[q] - xc[q,m])( = -g1 )
Blocks 2,3 of the reference concat are duplicates that can never win argmin
(first occurrence wins), so only d0/d1 are searched.

Device pipeline per core (codebook axis M sharded 8 ways, 2016 rows/core):
  * fp16 GEMM psum[q,m] = sum_d 2*xq[d,q]*c[d,m] - c2[m]: the -c2 term is
    folded into contraction k-tile 0 as two exact fp16 hi/lo rows, so PSUM
    directly holds g0 = 2xc - c2.
  * Loop nest is m-chunk(504) OUTER, k-tile MID, q-tile INNER: the 8
    q-tile accumulators occupy all 8 PSUM banks, each ct k-tile chunk
    (127KB) feeds 8 back-to-back matmuls, and the DMA stream (few large
    transfers, issued in exact consumption order on the SP HWDGE ring)
    stays ~3us ahead of the PE with no mid-kernel stalls. The PE starts
    after ~390KB of input instead of ~8MB.
  * Per (chunk, qt) as soon as its k-accumulation stops:
    ACT stages PSUM->SBUF; DVE: v = (-t2) - srep (scalar_tensor_tensor),
    h = (v + 2xs) max t2  -> h[m] = max(g0, g1) = -(min(d0,d1) - x2),
    then top-8 value/index per 504-wide chunk. Only the last chunk's last
    q-tile chain sits in the kernel tail.
  * host merges the 32 candidates (max value, lowest-index tie-break),
    recovers which of d0/d1 won with one exact f64 dot per query, and
    emits the bits.
"""

import numpy as np

import concourse.bass as bass
import concourse.tile as tile
from concourse import bacc, mybir
from concourse.bass_utils import run_bass_kernel_spmd

# Problem constants (hardcoded per the harness contract).
B = 64
KSLOT = 16
D = 2016
M = 16001
NBITS = 32
BK = B * KSLOT           # 1024 queries
NCORES = 8
MLOC = 2004              # per-core codebook rows (8*2004 >= 16001)
NCH = 4                  # m-chunks per core
CW = MLOC // NCH         # 501 columns per chunk = one PSUM bank (<=512 f32)
CWA = (CW + 1) // 2      # last q-tile is accumulated as two half-chunks
CWB = CW - CWA           # (251 + 250) so the kernel-tail DVE chain halves
KT = 126                 # contraction rows per k-tile (16*126 = 2016)
NK = D // KT             # 16 k-tiles; every tile padded to 128 partitions
NQT = BK // 128          # 8 query tiles
NSLOT = NCH * NQT + 1    # candidate slots: 32 regular + last-qt second half
PAD_C2HI = np.float16(60000.0)   # g0 for padded codes ~ -60000: never wins
PAD_BIG = np.float32(1e30)       # srep padding: g1 ~ -1e30: never wins

# DMA grouping (k-tiles per transfer) for the xq and chunk-0 ct streams:
# fine-grained at the front so the first matmul starts after ~390KB.
GROUPS = ((0, 1), (1, 2), (2, 4), (4, 6), (6, 8), (8, 10), (10, 12),
          (12, 14), (14, 16))
GROUPS2 = ((0, 8), (8, 16))

_compiled = {}


def _build_program(repeat: int = 1) -> bass.Bass:
    """repeat>1 replays the whole body (DMAs + compute) N times inside one
    NEFF — used by test.py to measure per-iteration device time
    differentially (dispatch overhead cancels)."""
    f16 = mybir.dt.float16
    f32 = mybir.dt.float32
    u32 = mybir.dt.uint32

    nc = bacc.Bacc("TRN2", debug=False, num_devices=NCORES)

    # Unused input: bench.py passes a distinct nonce per chained execution
    # so identical back-to-back custom calls can't be CSE'd by XLA.
    nc.dram_tensor("nonce", [1, 1], f32, kind="ExternalInput")
    # xqt: [128, NK*BK] — col k*BK+q is query q of k-tile k; rows 126,127 of
    # k-tile 0 are the two aug (ones) rows, zero-padding rows elsewhere.
    xqt = nc.dram_tensor("xqt", [128, NK * BK], f16, kind="ExternalInput").ap()
    # ct: [NCH][128, NK*CW] — col k*CW+j is code column j of chunk c, k-tile
    # k; rows 126,127 of k-tile 0 carry the -c2 hi/lo rows.
    ct = nc.dram_tensor("ct", [NCH, 128, NK * CW], f16, kind="ExternalInput").ap()
    srep = nc.dram_tensor("srep", [128, MLOC], f32, kind="ExternalInput").ap()
    xs2 = nc.dram_tensor("xs2", [128, NQT], f32, kind="ExternalInput").ap()
    outv = nc.dram_tensor("outv", [128, NSLOT * 8], f32,
                          kind="ExternalOutput").ap()
    outi = nc.dram_tensor("outi", [128, NSLOT * 8], u32,
                          kind="ExternalOutput").ap()

    import contextlib

    with tile.TileContext(nc) as tc:
        with (
            tc.tile_pool(name="ins", bufs=1) as in_pool,
            tc.tile_pool(name="psum", bufs=8, space="PSUM") as psum_pool,
            tc.tile_pool(name="work", bufs=12) as work_pool,
            tc.tile_pool(name="outs", bufs=2) as out_pool,
        ):
          # repeat>1: dynamic loop (body emitted once — the NEFF stays small
          # and each iteration is separated by the loop's full barrier, so
          # per-iteration wall ~= one-shot exec minus fixed NEFF overheads).
          loop = (tc.For_i(0, repeat, 1,
                           hint_engines=(mybir.EngineType.PE,
                                         mybir.EngineType.DVE,
                                         mybir.EngineType.Activation,
                                         mybir.EngineType.SP))
                  if repeat > 1 else contextlib.nullcontext())
          with loop:
            xs2_t = in_pool.tile([128, NQT], f32, tag="xs2")
            xq_t = in_pool.tile([128, NK * BK], f16, tag="xq")
            ct_t = []
            for c in range(NCH):
                ctc = in_pool.tile([128, NK * CW], f16, tag=f"ct{c}")
                ct_t.append(ctc)
            srep_t = in_pool.tile([128, MLOC], f32, tag="srep")
            ov_t = in_pool.tile([128, NSLOT * 8], f32, tag="ov")
            oi_t = in_pool.tile([128, NSLOT * 8], u32, tag="oi")
            warm_t = in_pool.tile([128, 128], f16, tag="warm")

            # Input stream, issued in exact consumption order on one ring.
            # First two pieces are exactly the first matmul's operands
            # (~160KB): qt0's k0 weights, then ct chunk0 k0.
            nc.sync.dma_start(xq_t[:, 0:128], xqt[:, 0:128])
            nc.sync.dma_start(ct_t[0][:, 0:CW], ct[0, :, 0:CW])
            nc.sync.dma_start(xq_t[:, 128:BK], xqt[:, 128:BK])
            for s, e in GROUPS[1:]:
                nc.sync.dma_start(xq_t[:, s * BK:e * BK],
                                  xqt[:, s * BK:e * BK])
                nc.sync.dma_start(ct_t[0][:, s * CW:e * CW],
                                  ct[0, :, s * CW:e * CW])
            nc.sync.dma_start(xs2_t[:], xs2[:, :])
            nc.sync.dma_start(srep_t[:], srep[:, :])
            for c in range(1, NCH):
                for s, e in GROUPS2:
                    nc.sync.dma_start(ct_t[c][:, s * CW:e * CW],
                                      ct[c, :, s * CW:e * CW])

            def post(c, qt, ps, slot, off=0, w=CW, direct=False):
                """PSUM->SBUF stage + DVE distance-combine + top-8 into
                candidate slot `slot` (index base = c*CW + off).

                direct=True reads PSUM straight from the DVE (slower access
                but drops the serial ACT copy) — used only for the very
                last q-tile halves, whose chains ARE the kernel tail."""
                if direct:
                    t2 = ps[:]
                else:
                    t2 = work_pool.tile([128, CW], f32, tag="t2")
                    t2 = t2[:, 0:w]
                    nc.scalar.copy(t2, ps[:])
                v = work_pool.tile([128, CW], f32, tag="v")
                nc.vector.scalar_tensor_tensor(
                    v[:, 0:w], in0=t2, scalar=-1.0,
                    in1=srep_t[:, c * CW + off:c * CW + off + w],
                    op0=mybir.AluOpType.mult,
                    op1=mybir.AluOpType.subtract,
                )
                h = work_pool.tile([128, CW], f32, tag="h")
                nc.vector.scalar_tensor_tensor(
                    h[:, 0:w], in0=v[:, 0:w], scalar=xs2_t[:, qt:qt + 1],
                    in1=t2,
                    op0=mybir.AluOpType.add,
                    op1=mybir.AluOpType.max,
                )
                o = slot * 8
                nc.vector.max(ov_t[:, o:o + 8], h[:, 0:w])
                nc.vector.max_index(
                    oi_t[:, o:o + 8], ov_t[:, o:o + 8], h[:, 0:w])

            def mm(c, k, qt, ps, off=0, w=CW):
                nc.tensor.matmul(
                    ps[:],
                    lhsT=xq_t[:, k * BK + qt * 128:k * BK + (qt + 1) * 128],
                    rhs=ct_t[c][:, k * CW + off:k * CW + off + w],
                    start=(k == 0),
                    stop=(k == NK - 1),
                )

            def ship(slots, sl0):
                """DMA candidate slots [sl0, sl0+slots) to DRAM."""
                o = sl0 * 8
                n = slots * 8
                nc.sync.dma_start(outv[:, o:o + n], ov_t[:, o:o + n])
                nc.sync.dma_start(outi[:, o:o + n], oi_t[:, o:o + n])

            # PE warm-up: ~24 matmuls on a zeroed scratch tile keep the PE
            # HAM activity window busy while the first input DMAs land, so
            # the real matmul stream starts closer to full clock. Results
            # land in the first accumulator and are cleared by its real
            # start=True matmul.
            nc.vector.memset(warm_t[:], 0.0)
            warm_ps = psum_pool.tile([128, CW], f32, tag="ps")
            for _ in range(24):
                nc.tensor.matmul(warm_ps[:, 0:128], lhsT=warm_t[:],
                                 rhs=warm_t[:], start=True, stop=True)

            for c in range(NCH):
                lastc = (c == NCH - 1)
                pss = []
                for _ in range(NQT - 1 if lastc else NQT):
                    ps = psum_pool.tile([128, CW], f32, tag="ps")
                    pss.append(ps)
                if c == 0:
                    # Streaming chunk: k OUTER so each arriving ct k-tile
                    # feeds 8 back-to-back matmuls — the PE starts after
                    # ~390KB of DMA. The 8 post-chains burst at chunk end
                    # and overlap chunk 1's matmuls.
                    for k in range(NK):
                        for qt in range(NQT):
                            mm(c, k, qt, pss[qt])
                            if k == NK - 1:
                                post(c, qt, pss[qt], c * NQT + qt)
                    ship(NQT, c * NQT)
                elif not lastc:
                    # Resident chunks: qt OUTER so stop-matmuls spread
                    # every 16 matmuls.
                    for qt in range(NQT):
                        for k in range(NK):
                            mm(c, k, qt, pss[qt])
                        post(c, qt, pss[qt], c * NQT + qt)
                    ship(NQT, c * NQT)
                else:
                    # Last chunk: last q-tile accumulated as two half-width
                    # groups so only a half-width DVE chain trails the final
                    # matmul; candidates shipped per q-tile.
                    for qt in range(NQT - 1):
                        for k in range(NK):
                            mm(c, k, qt, pss[qt])
                        post(c, qt, pss[qt], c * NQT + qt)
                        ship(1, c * NQT + qt)
                    qt = NQT - 1
                    psa = psum_pool.tile([128, CWA], f32, tag="ps")
                    psb = psum_pool.tile([128, CWB], f32, tag="ps")
                    for k in range(NK):
                        mm(c, k, qt, psa, off=0, w=CWA)
                    for k in range(NK):
                        mm(c, k, qt, psb, off=CWA, w=CWB)
                    post(c, qt, psa, c * NQT + qt, off=0, w=CWA, direct=True)
                    ship(1, c * NQT + qt)
                    post(c, qt, psb, NSLOT - 1, off=CWA, w=CWB, direct=True)
                    ship(1, NSLOT - 1)

    nc.compile()
    return nc


def _host_prep(x: np.ndarray, data: np.ndarray):
    """Build per-core input maps: layout/shard prep plus the tiny norm
    vectors (c2/cn2 sums); all heavy FLOPs stay on device."""
    xq = np.transpose(
        x.reshape(B, 2, 126, KSLOT, 8), (0, 3, 1, 2, 4)
    ).reshape(BK, D)
    # xqt: [128, NK*BK]; k-tile k rows 0:126 = (2*xq).T rows of that k-tile;
    # k-tile 0 rows 126,127 are the aug coefficient rows (ones).
    xqt2 = np.zeros((128, NK * BK), dtype=np.float16)
    xq2T = (xq.astype(np.float16) * np.float16(2.0)).T   # exact fp16 scaling
    for k in range(NK):
        xqt2[0:KT, k * BK:(k + 1) * BK] = xq2T[k * KT:(k + 1) * KT]
    xqt2[KT:128, 0:BK] = 1.0

    xq64 = xq.astype(np.float64)
    xs2 = np.ascontiguousarray(
        (2.0 * xq64.sum(axis=1)).astype(np.float32).reshape(NQT, 128).T
    )

    c = data.reshape(M, D)
    c64 = c.astype(np.float64)
    c2_all = np.einsum("md,md->m", c64, c64)
    # cn2 = sum((1-c)^2) = D - 2*sum(c) + c2, exact in f64.
    cn2_all = D - 2.0 * c64.sum(axis=1) + c2_all

    in_maps = []
    for core in range(NCORES):
        s = core * MLOC
        e = min(s + MLOC, M)
        n = e - s
        cloc = np.zeros((MLOC, D), dtype=np.float16)
        cloc[:n] = c[s:e]
        # Exact fp16 hi/lo split of -c2 in the two aug rows.
        c2_hi = np.full(MLOC, -PAD_C2HI, dtype=np.float16)
        c2_hi[:n] = -c2_all[s:e].astype(np.float16)
        c2_lo = np.zeros(MLOC, dtype=np.float16)
        c2_lo[:n] = -(c2_all[s:e] + c2_hi[:n].astype(np.float64))
        # ct: [NCH, 128, NK*CW]
        ctl = np.zeros((NCH, 128, NK * CW), dtype=np.float16)
        for cch in range(NCH):
            blk = cloc[cch * CW:(cch + 1) * CW]              # [CW, D]
            for k in range(NK):
                ctl[cch, 0:KT, k * CW:(k + 1) * CW] = \
                    blk[:, k * KT:(k + 1) * KT].T
            ctl[cch, KT, 0:CW] = c2_hi[cch * CW:(cch + 1) * CW]
            ctl[cch, KT + 1, 0:CW] = c2_lo[cch * CW:(cch + 1) * CW]
        sloc = np.full(MLOC, PAD_BIG, dtype=np.float32)
        sloc[:n] = (c2_all[s:e] + cn2_all[s:e]).astype(np.float32)
        in_maps.append({
            "nonce": np.zeros((1, 1), dtype=np.float32),
            "xqt": xqt2,
            "ct": ctl,
            "srep": np.ascontiguousarray(
                np.broadcast_to(sloc[None, :], (128, MLOC))
            ),
            "xs2": xs2,
        })
    return in_maps


def _merge(results, x: np.ndarray, data: np.ndarray):
    """Merge per-core top-1 candidates; recover the d0/d1 side with one
    exact f64 dot per query."""
    # outv/outi: [128, NSLOT*8]; slot s<NCH*NQT covers chunk s//NQT (base
    # (s//NQT)*CW) for q-tile s%NQT; the extra slot NSLOT-1 is the second
    # half (base (NCH-1)*CW + CWA) of the last chunk for q-tile NQT-1.
    # Query q = (s%NQT)*128 + p. Top-1 of each slot only.
    vals = np.stack([r["outv"].reshape(128, NSLOT, 8)[:, :, 0]
                     for r in results])                      # [8,128,NSLOT]
    ms = np.stack(
        [r["outi"].reshape(128, NSLOT, 8)[:, :, 0].astype(np.int64)
         for r in results]
    )
    base = np.concatenate([
        np.repeat(np.arange(NCH, dtype=np.int64) * CW, NQT),
        [(NCH - 1) * CW + CWA],
    ])                                                       # [NSLOT]
    qt_of = np.concatenate([
        np.tile(np.arange(NQT, dtype=np.int64), NCH), [NQT - 1]])
    ms = ms + base[None, None, :]
    ms = ms + np.arange(NCORES, dtype=np.int64).reshape(NCORES, 1, 1) * MLOC

    # Per query: candidates = slots whose q-tile matches, ordered by
    # ascending global base (ties: lowest global index wins via
    # argmax-first-occurrence). Per-core slot order by base: chunks 0..3
    # then the extra half — already ascending; cores ascend outermost.
    r_win = np.empty(BK, dtype=np.int64)
    for qt in range(NQT):
        sl = np.where(qt_of == qt)[0]
        order = sl[np.argsort(base[sl], kind="stable")]
        v_q = vals[:, :, order].transpose(0, 2, 1).reshape(-1, 128)
        m_q = ms[:, :, order].transpose(0, 2, 1).reshape(-1, 128)
        b = np.argmax(v_q, axis=0)
        r_win[qt * 128:(qt + 1) * 128] = m_q[b, np.arange(128)]

    xq = np.transpose(
        x.reshape(B, 2, 126, KSLOT, 8), (0, 3, 1, 2, 4)
    ).reshape(BK, D).astype(np.float64)
    cwin = data.reshape(M, D)[r_win].astype(np.float64)             # [1024,D]
    dot = np.einsum("qd,qd->q", xq, cwin)
    xs = xq.sum(axis=1)
    # d0 - d1 = c2 - cn2 - 2*(2*dot - xs); side 0 wins ties.
    c2 = (cwin * cwin).sum(axis=1)
    cn2 = ((1.0 - cwin) ** 2).sum(axis=1)
    side = (c2 - 2.0 * dot > cn2 - 2.0 * (xs - dot)).astype(np.int64)
    return r_win + side * M                                         # [1024]


def kernel(x: np.ndarray, data: np.ndarray) -> np.ndarray:
    if "nc" not in _compiled:
        _compiled["nc"] = _build_program()
    nc = _compiled["nc"]

    x = np.asarray(x)
    data = np.asarray(data)
    in_maps = _host_prep(x, data)
    res = run_bass_kernel_spmd(nc, in_maps, list(range(NCORES)))
    _compiled["last_result"] = res

    # Candidate indices within [0, MLOC) per (core, chunk, qt); queries are
    # qt*128+p. _merge handles global-row/tie-break/side recovery.
    g = _merge(res.results, x, data).astype(np.int32)               # [1024]
    shifts = np.arange(NBITS, dtype=np.int32)
    bits = (g[:, None] >> shifts[None, :]) & 1
    return bits.astype(np.int32).reshape(B, KSLOT * NBITS)
